# revision 2
# baseline (speedup 1.0000x reference)
"""Trainium2 Bass kernel for nn_Custom_trainer_79242146611896 — v3.

Data-parallel over N=16384 samples across 8 NeuronCores (2048/core).
v3 design (vs v2 baseline, 284us):
  - host packs x TRANSPOSED as fp8 DoubleRow pair tiles: kills all 256
    x PE-transposes (~70us PE) and 12MB/core of DMA.
  - host packs (o^T - b_dec) x16 as fp8 [T,n] tiles: mm2 emits decodedT
    so the pinball subtract is elementwise vs oT8; 12MB/core DMA saved.
  - weights pre-cast on host (fp8 DR pairs / bf16), cat_labels packed
    bf16 [128, NN*C]: no device-side CAST passes, ~7MB/core DMA saved.
    Total DMA ~10.7MB/core (was ~41.5MB).
  - collective #2 removed: each core returns its 3 loss partials in a
    padded output row; host computes the global scalar during gather.
    Kills the ~24us exposed AllReduce tail + ~15us drain.
  - one collective (seg sums+counts [C,D+1]) launched at end of phase
    A, hidden under phase B (mm2/mm4/mm3 for all chunks).
  - pinball elementwise work split across vector/gpsimd/scalar.
"""

import numpy as np
import ml_dtypes

import concourse.bass as bass
import concourse.mybir as mybir
import concourse.tile as tile
from concourse import bacc
from concourse.bass_utils import run_bass_kernel_spmd
from concourse.masks import make_identity

F32 = mybir.dt.float32
FP8 = mybir.dt.float8e4
BF16 = mybir.dt.bfloat16
I32 = mybir.dt.int32
AX = mybir.AxisListType
ALU = mybir.AluOpType
ACTF = mybir.ActivationFunctionType
DROW = mybir.MatmulPerfMode.DoubleRow

P = 128
NCORES = 8
N_GLOBAL = 16384
T = 2048
D = 512
C = 50
KEPS = 1e-7
LNEPS = float(np.log(KEPS))

NL = N_GLOBAL // NCORES   # 2048 samples per core
NT = T // P               # 16 T-tiles
ND = D // P               # 4 D-tiles
NN = NL // P              # 16 sample-tiles per core
NC = 512                  # samples per chunk
NCH = NL // NC            # 4 chunks
NSUB = NC // P            # 4 sample-tiles per chunk

WENC_SC = 32.0            # wenc fp8 scale (tanh un-scales via 1/32)
DSC = 16.0                # wdec/Wdd/o fp8 scale


def build(gps_b=True):
    nc = bacc.Bacc("TRN2", target_bir_lowering=False, debug=False, num_devices=NCORES)

    # host-packed inputs
    xt8_d = nc.dram_tensor("xt8", [NCH * NT // 2 * P, 2 * NC], FP8, kind="ExternalInput")
    ot8_d = nc.dram_tensor("ot8", [NT * P, NL], FP8, kind="ExternalInput")
    wenc8_d = nc.dram_tensor("wenc8", [NT // 2 * P, 2 * D], FP8, kind="ExternalInput")
    wdec8_d = nc.dram_tensor("wdec8", [ND // 2 * P, 2 * T], FP8, kind="ExternalInput")
    wdd8_d = nc.dram_tensor("wdd8", [ND // 2 * P, 2 * D], FP8, kind="ExternalInput")
    wcls_d = nc.dram_tensor("wcls16", [D, C], BF16, kind="ExternalInput")
    cat_d = nc.dram_tensor("cat16", [P, NN * C], BF16, kind="ExternalInput")
    ohn_d = nc.dram_tensor("ohn16", [P, NN * C], BF16, kind="ExternalInput")
    oht_d = nc.dram_tensor("oht16", [C, NL], BF16, kind="ExternalInput")
    benc_d = nc.dram_tensor("b_enc", [D], F32, kind="ExternalInput")
    bcls_d = nc.dram_tensor("b_cls16", [C], BF16, kind="ExternalInput")
    out_d = nc.dram_tensor("out", [NL + P], F32, kind="ExternalOutput")

    from contextlib import ExitStack

    with tile.TileContext(nc) as tc:
        with ExitStack() as ctx:
            ent = ctx.enter_context
            constp = ent(tc.tile_pool(name="const", bufs=1))
            wts = ent(tc.tile_pool(name="wts", bufs=1))
            bigp = ent(tc.tile_pool(name="big", bufs=1))
            encp = ent(tc.tile_pool(name="enc", bufs=1))
            accp = ent(tc.tile_pool(name="acc", bufs=1))
            enp = ent(tc.tile_pool(name="enp", bufs=6))
            junkp = ent(tc.tile_pool(name="junk", bufs=4))
            ccep = ent(tc.tile_pool(name="cce", bufs=4))
            smallp = ent(tc.tile_pool(name="small", bufs=10))
            psA = ent(tc.tile_pool(name="psA", bufs=2, space="PSUM"))
            psT = ent(tc.tile_pool(name="psT", bufs=2, space="PSUM"))
            psS = ent(tc.tile_pool(name="psS", bufs=2, space="PSUM"))
            psSeg = ent(tc.tile_pool(name="psSeg", bufs=1, space="PSUM"))
            psCnt = ent(tc.tile_pool(name="psCnt", bufs=1, space="PSUM"))
            dp = ent(tc.tile_pool(name="dram", bufs=1, space="DRAM"))

            # ------- big input slabs: phase-A-critical DMAs first -------
            wenc8 = wts.tile([P, NT // 2 * 2 * D], FP8)
            nc.sync.dma_start(
                wenc8[:].rearrange("p (a m) -> p a m", a=NT // 2),
                wenc8_d.ap().rearrange("(a p) m -> p a m", p=P),
            )
            # xt8 chunk-major: one 1MB DMA per sample-chunk so mm1 on
            # chunk 0 starts as soon as the first quarter lands.
            xt8 = bigp.tile([P, NCH * NT // 2 * 2 * NC], FP8)
            xt8_src = xt8_d.ap().rearrange("(c a p) m -> c p a m", p=P, c=NCH)
            CW = NT // 2 * 2 * NC
            for ch in range(NCH):
                nc.sync.dma_start(
                    xt8[:, ch * CW : (ch + 1) * CW].rearrange(
                        "p (a m) -> p a m", a=NT // 2
                    ),
                    xt8_src[ch],
                )
            bencT = wts.tile([P, ND], F32)
            nc.sync.dma_start(bencT[:], benc_d.ap().rearrange("(a p) -> p a", p=P))
            ohnb = bigp.tile([P, NN * C], BF16)
            nc.sync.dma_start(ohnb[:], ohn_d.ap())
            ohtb = bigp.tile([C, NL], BF16)
            nc.sync.dma_start(ohtb[:], oht_d.ap())
            bcls_row_t = wts.tile([1, C], BF16)
            bcls_row = bcls_row_t[0:1, :]
            nc.sync.dma_start(bcls_row, bcls_d.ap().rearrange("(o c) -> o c", o=1))
            wclsb = wts.tile([P, ND * C], BF16)
            nc.sync.dma_start(
                wclsb[:].rearrange("p (a c) -> p a c", a=ND),
                wcls_d.ap().rearrange("(a p) c -> p a c", p=P),
            )
            catb = bigp.tile([P, NN * C], BF16)
            nc.sync.dma_start(catb[:], cat_d.ap())
            wdd8 = wts.tile([P, ND // 2 * 2 * D], FP8)
            nc.sync.dma_start(
                wdd8[:].rearrange("p (a m) -> p a m", a=ND // 2),
                wdd8_d.ap().rearrange("(a p) m -> p a m", p=P),
            )
            wdec8 = wts.tile([P, ND // 2 * 2 * T], FP8)
            nc.sync.dma_start(
                wdec8[:].rearrange("p (a m) -> p a m", a=ND // 2),
                wdec8_d.ap().rearrange("(a p) m -> p a m", p=P),
            )
            # ot8 last: the sync HWDGE ring drains FIFO, so emission
            # order prioritizes the phase-A-critical loads above.
            ot8 = bigp.tile([P, NT * NL], FP8)
            nc.sync.dma_start(
                ot8[:].rearrange("p (a m) -> p a m", a=NT),
                ot8_d.ap().rearrange("(a p) m -> p a m", p=P),
            )

            # ---------------- constants ----------------
            ident_f32 = constp.tile([P, P], F32)
            make_identity(nc, ident_f32)
            ident_bf = constp.tile([P, P], BF16)
            nc.vector.tensor_copy(ident_bf[:], ident_f32[:])
            nident8 = constp.tile([P, P], FP8)
            nc.scalar.activation(nident8[:], ident_f32[:], mybir.ActivationFunctionType.Copy, scale=-1.0)
            ones_col_f = constp.tile([P, 1], F32)
            nc.any.memset(ones_col_f[:], 1.0)
            ones_col_bf = constp.tile([P, 1], BF16)
            nc.vector.tensor_copy(ones_col_bf[:], ones_col_f[:])
            ones_k1bf = constp.tile([1, P], BF16)
            nc.any.memset(ones_k1bf[:], 1.0)

            # pre-touch the tanh table during the DMA wait
            dummy = smallp.tile([P, 1], F32, name="dummy", tag="dummy")
            nc.scalar.activation(dummy[:], ones_col_f[:], ACTF.Tanh)

            # ---------------- persistent state ----------------
            encT = [encp.tile([P, NL], BF16, name=f"encT{k}", tag=f"encT{k}") for k in range(ND)]
            encT8 = [encp.tile([P, 2 * NL], FP8, name=f"encT8_{kp}", tag=f"encT8_{kp}") for kp in range(ND // 2)]
            rec_strip = accp.tile([P, NCH * NT], F32)
            lat_strip = accp.tile([P, NCH * ND], F32)
            cat_strip = accp.tile([P, NN], F32)
            nsq_strip = accp.tile([P, NN], F32)

            seg_ps = psSeg.tile([C, D], F32, name="seg_ps", tag="seg")
            cnt_ps = psCnt.tile([C, 1], F32, name="cnt_ps", tag="cnt")

            xt8_r = xt8[:].rearrange(
                "p (c a two n) -> p c a two n", c=NCH, a=NT // 2, two=2
            )
            wenc8_r = wenc8[:].rearrange("p (a two d) -> p a two d", a=NT // 2, two=2)
            wdec8_r = wdec8[:].rearrange("p (a two t) -> p a two t", a=ND // 2, two=2)
            wdd8_r = wdd8[:].rearrange("p (a two d) -> p a two d", a=ND // 2, two=2)

            # ================= phase A: mm1 + transposes + seg =================
            for c in range(NCH):
                base = c * NC
                for k in range(ND):
                    ps = psA.tile([P, NC], F32, name="ps1", tag="psA")
                    for tp in range(NT // 2):
                        nc.tensor.matmul(
                            ps[:],
                            wenc8_r[:, tp, :, k * P : (k + 1) * P],
                            xt8_r[:, c, tp, :, :],
                            start=(tp == 0), stop=(tp == NT // 2 - 1),
                            perf_mode=DROW,
                        )
                    nc.scalar.activation(
                        encT[k][:, base : base + NC], ps[:], ACTF.Tanh,
                        bias=bencT[:, k : k + 1], scale=1.0 / WENC_SC,
                    )
                    nc.gpsimd.tensor_copy(
                        encT8[k // 2][:, (k % 2) * NL + base : (k % 2) * NL + base + NC],
                        encT[k][:, base : base + NC],
                    )

                for s in range(NSUB):
                    i = c * NSUB + s
                    pse = psT.tile([P, D], BF16, name="pseb", tag="psT")
                    for k in range(ND):
                        nc.tensor.transpose(
                            pse[:, k * P : (k + 1) * P],
                            encT[k][:, base + s * P : base + (s + 1) * P],
                            ident_bf,
                        )
                    en = enp.tile([P, D], BF16, name="en", tag="en")
                    nc.scalar.copy(en[:], pse[:])

                    oh_i = ohnb[:, i * C : (i + 1) * C]
                    nc.tensor.matmul(
                        seg_ps[:], oh_i, en[:],
                        start=(i == 0), stop=(i == NN - 1),
                    )
                    nc.tensor.matmul(
                        cnt_ps[:], oh_i, ones_col_bf[:],
                        start=(i == 0), stop=(i == NN - 1),
                    )

                    jn = junkp.tile([P, D], F32, name="jn", tag="junk")
                    nc.vector.scalar_tensor_tensor(
                        out=jn[:], in0=en[:], scalar=0.0, in1=en[:],
                        op0=ALU.bypass, op1=ALU.mult,
                        accum_out=nsq_strip[:, i : i + 1],
                    )

            # ============ collective: seg sums + counts ============
            seg_sb = accp.tile([C, D], F32)
            nc.vector.tensor_copy(seg_sb[:], seg_ps[:])
            cnt_sb = accp.tile([C, 1], F32)
            nc.vector.tensor_copy(cnt_sb[:], cnt_ps[:])

            bounce1_in = dp.tile([C, D + 1], F32, name="bounce1_in", tag="b1i")
            bounce1_out = dp.tile([C, D + 1], F32, name="bounce1_out", tag="b1o")
            nc.sync.dma_start(bounce1_in[:, 0:D], seg_sb[:])
            nc.sync.dma_start(bounce1_in[:, D : D + 1], cnt_sb[:])
            nc.gpsimd.collective_compute(
                "AllReduce",
                ALU.add,
                replica_groups=[list(range(NCORES))],
                ins=[bounce1_in[:].opt()],
                outs=[bounce1_out[:].opt()],
            )

            # ================= phase B (interleaved per chunk: B1's
            # matmuls cover B2/B3's vector+scalar work) =================
            for c in range(NCH):
                base = c * NC
                # B1: decodedT per T-tile + rec pinball (diff is 16x scaled;
                # host coefficient divides by 16). Odd tiles subtract o on
                # the PE via a -I @ oT8 accumulation matmul.
                for t in range(NT):
                    pool = (psA, psT, psS)[t % 3]
                    ps = pool.tile([P, NC], F32, name="ps2", tag=pool.name)
                    for kp in range(ND // 2):
                        nc.tensor.matmul(
                            ps[:],
                            wdec8_r[:, kp, :, t * P : (t + 1) * P],
                            encT8[kp].rearrange("p (two n) -> p two n", two=2)[
                                :, :, base : base + NC
                            ],
                            start=(kp == 0),
                            stop=(kp == ND // 2 - 1 and t % 2 == 0),
                            perf_mode=DROW,
                        )
                    if t % 2 == 1:
                        nc.tensor.matmul(
                            ps[:], nident8[:],
                            ot8[:, t * NL + base : t * NL + base + NC],
                            start=False, stop=True,
                        )
                    if t % 2 == 0:
                        nc.vector.tensor_tensor(
                            ps[:], ps[:], ot8[:, t * NL + base : t * NL + base + NC],
                            ALU.subtract,
                        )
                    col = rec_strip[:, c * NT + t : c * NT + t + 1]
                    if t % 4 < 2:
                        nc.scalar.activation(ps[:], ps[:], ACTF.Abs, accum_out=col)
                    else:
                        nc.vector.tensor_reduce(
                            col, ps[:], AX.X, ALU.add, apply_absolute_value=True,
                        )

                # B2: rec_latentsT via W_dd + lat pinball
                for k2 in range(ND):
                    pool = (psA, psT, psS)[(NT + k2) % 3]
                    ps = pool.tile([P, NC], F32, name="ps4", tag=pool.name)
                    for kp in range(ND // 2):
                        nc.tensor.matmul(
                            ps[:],
                            wdd8_r[:, kp, :, k2 * P : (k2 + 1) * P],
                            encT8[kp].rearrange("p (two n) -> p two n", two=2)[
                                :, :, base : base + NC
                            ],
                            start=(kp == 0), stop=(kp == ND // 2 - 1),
                            perf_mode=DROW,
                        )
                    nc.scalar.activation(
                        ps[:], ps[:], ACTF.Tanh, bias=bencT[:, k2 : k2 + 1], scale=1.0 / DSC
                    )
                    nc.vector.tensor_tensor(
                        ps[:], ps[:], encT[k2][:, base : base + NC], ALU.subtract
                    )
                    col = lat_strip[:, c * ND + k2 : c * ND + k2 + 1]
                    if k2 % 2 == 0:
                        nc.scalar.activation(ps[:], ps[:], ACTF.Abs, accum_out=col)
                    else:
                        nc.vector.tensor_reduce(
                            col, ps[:], AX.X, ALU.add, apply_absolute_value=True,
                        )

                # B3: logits + CCE
                psl = psS.tile([P, NSUB * C], F32, name="psl", tag="psS")
                for a in range(NSUB):
                    sl = slice(a * C, (a + 1) * C)
                    nc.tensor.matmul(
                        psl[:, sl], ones_k1bf[:], bcls_row[:],
                        start=True, stop=False,
                    )
                    for k in range(ND):
                        nc.tensor.matmul(
                            psl[:, sl],
                            encT[k][:, (base + a * P) : (base + (a + 1) * P)],
                            wclsb[:, k * C : (k + 1) * C],
                            start=False, stop=(k == ND - 1),
                        )
                expt = ccep.tile([P, NSUB * C], F32, name="expt", tag="cce", bufs=2)
                nc.scalar.activation(expt[:], psl[:], ACTF.Exp)
                sume = smallp.tile([P, NSUB], F32, name="sume", tag="small")
                nc.vector.tensor_reduce(
                    sume[:], expt[:].rearrange("p (a c) -> p a c", c=C),
                    AX.X, ALU.add,
                )
                rcp = smallp.tile([P, NSUB], F32, name="rcp", tag="small")
                nc.vector.reciprocal(rcp[:], sume[:])
                clsl = catb[:, c * NSUB * C : (c + 1) * NSUB * C]
                rs = smallp.tile([P, NSUB], F32, name="rs", tag="small")
                nc.vector.tensor_reduce(
                    rs[:], clsl.rearrange("p (a c) -> p a c", c=C),
                    AX.X, ALU.add,
                )
                lnrs = smallp.tile([P, NSUB], F32, name="lnrs", tag="small")
                nc.scalar.activation(lnrs[:], rs[:], ACTF.Ln)
                lg = ccep.tile([P, NSUB * C], F32, name="lg", tag="lgbuf", bufs=2)
                nc.scalar.activation(lg[:], clsl, ACTF.Ln)
                for a in range(NSUB):
                    i = c * NSUB + a
                    sl = slice(a * C, (a + 1) * C)
                    lgc = ccep.tile([P, C], F32, name="lgc", tag="lgc")
                    nc.vector.tensor_scalar(
                        out=lgc[:], in0=lg[:, sl], scalar1=lnrs[:, a : a + 1],
                        scalar2=LNEPS, op0=ALU.subtract, op1=ALU.max,
                    )
                    jc = ccep.tile([P, C], F32, name="jc", tag="lgc")
                    nc.vector.scalar_tensor_tensor(
                        out=jc[:], in0=expt[:, sl], scalar=rcp[:, a : a + 1],
                        in1=lgc[:], op0=ALU.mult, op1=ALU.mult,
                        accum_out=cat_strip[:, i : i + 1],
                    )

            # readback of collective (late emission keeps the DMA queue clear)
            sums_g = accp.tile([C, D], F32)
            nc.sync.dma_start(sums_g[:], bounce1_out[:, 0:D])
            counts_g = accp.tile([C, 1], F32)
            nc.sync.dma_start(counts_g[:], bounce1_out[:, D : D + 1])

            # ================= phase C =================
            cmax = accp.tile([C, 1], F32)
            nc.vector.tensor_scalar(
                out=cmax[:], in0=counts_g[:], scalar1=1.0, scalar2=None, op0=ALU.max
            )
            crcp = accp.tile([C, 1], F32)
            nc.vector.reciprocal(crcp[:], cmax[:])
            means = accp.tile([C, D], F32)
            nc.vector.tensor_scalar(
                out=means[:], in0=sums_g[:], scalar1=crcp[:], scalar2=None, op0=ALU.mult
            )
            msq_col = accp.tile([C, 1], F32)
            jm = junkp.tile([C, D], F32, name="jm", tag="junk")
            nc.vector.scalar_tensor_tensor(
                out=jm[:], in0=means[:], scalar=0.0, in1=means[:],
                op0=ALU.bypass, op1=ALU.mult, accum_out=msq_col[:],
            )

            meansT = []
            for k in range(ND):
                tpm = psS.tile([P, C], F32, name="tpm", tag="psS")
                nc.tensor.transpose(
                    tpm[:], means[:, k * P : (k + 1) * P], ident_f32[:C, :C]
                )
                mt = accp.tile([P, C], BF16, name="meansT", tag=f"meansT{k}")
                nc.vector.tensor_copy(mt[:], tpm[:])
                meansT.append(mt)

            # epsT[c,n] = sum_d means[c,d] enc[n,d]  (transposed layout);
            # jq2 = (epsT - 0.5*msq_col) * ohT ; gq_row[n] = colsum(jq2)
            msqh = accp.tile([C, 1], F32)
            nc.vector.tensor_scalar(
                out=msqh[:], in0=msq_col[:], scalar1=0.5, scalar2=None, op0=ALU.mult
            )
            gq_cols = psT.tile([P, NN], F32, name="gq_cols", tag="psT")
            for c in range(NCH):
                base = c * NC
                pscT = psS.tile([C, NC], F32, name="pscT", tag="psS")
                for k in range(ND):
                    nc.tensor.matmul(
                        pscT[:],
                        meansT[k][:],
                        encT[k][:, base : base + NC],
                        start=(k == 0), stop=(k == ND - 1),
                    )
                jq2 = ccep.tile([C, NC], F32, name="jq2", tag="jq2", bufs=2)
                nc.vector.scalar_tensor_tensor(
                    out=jq2[:], in0=pscT[:], scalar=msqh[:], in1=ohtb[:, base : base + NC],
                    op0=ALU.subtract, op1=ALU.mult,
                )
                gq_ps = psA.tile([1, NC], F32, name="gq_ps", tag="psA")
                nc.tensor.matmul(
                    gq_ps[:], ones_col_f[0:C, :], jq2[:], start=True, stop=True
                )
                gq_row = smallp.tile([1, NC], F32, name="gq_row", tag=f"gqr{c}", bufs=1)
                nc.vector.tensor_copy(gq_row[:], gq_ps[:])
                for s in range(NSUB):
                    i = c * NSUB + s
                    nc.tensor.transpose(
                        gq_cols[:, i : i + 1],
                        gq_row[:, s * P : (s + 1) * P],
                        ident_f32[0:1, 0:1],
                    )

            # ---- final: (nsq - 2*gq)/D per sample; pack partial scalars ----
            ns2 = accp.tile([P, NN], F32)
            nc.vector.tensor_scalar(
                out=ns2[:], in0=nsq_strip[:], scalar1=1.0 / D, scalar2=None, op0=ALU.mult
            )
            out_strip = accp.tile([P, NN], F32)
            nc.vector.scalar_tensor_tensor(
                out=out_strip[:], in0=gq_cols[:], scalar=-2.0 / D, in1=ns2[:],
                op0=ALU.mult, op1=ALU.add,
            )
            pack3 = accp.tile([P, 3], F32)
            nc.vector.tensor_reduce(pack3[:, 0:1], rec_strip[:], AX.X, ALU.add)
            nc.vector.tensor_reduce(pack3[:, 1:2], lat_strip[:], AX.X, ALU.add)
            nc.vector.tensor_reduce(pack3[:, 2:3], cat_strip[:], AX.X, ALU.add)
            scps = psS.tile([1, 3], F32, name="scps", tag="psS")
            nc.tensor.matmul(scps[:], ones_col_f[:], pack3[:], start=True, stop=True)

            ps_out = psS.tile([NN, P], F32, name="ps_out", tag="psS")
            nc.tensor.transpose(ps_out[:], out_strip[:], ident_f32)
            outT = accp.tile([NN, P], F32)
            nc.vector.tensor_copy(outT[:], ps_out[:])
            sc_row = accp.tile([1, P], F32)
            nc.any.memset(sc_row[:], 0.0)
            nc.vector.tensor_copy(sc_row[0:1, 0:3], scps[:])
            nc.sync.dma_start(
                out_d.ap()[0:NL].rearrange("(a p) -> a p", p=P), outT[:]
            )
            nc.sync.dma_start(
                out_d.ap()[NL : NL + P].rearrange("(o p) -> o p", o=1), sc_row[:]
            )

    nc.compile()
    return nc


_CACHE = {}


def _get_nc():
    if "nc" not in _CACHE:
        _CACHE["nc"] = build()
    return _CACHE["nc"]


FP8NP = ml_dtypes.float8_e4m3


def _pack_pairs(w, scale):
    """[K, M] f32 -> [K//256 * 128, 2*M] fp8 DoubleRow pair tiles."""
    K, M = w.shape
    a = K // (2 * P)
    r = (w * scale).reshape(a, 2, P, M).transpose(0, 2, 1, 3).reshape(a * P, 2 * M)
    return np.ascontiguousarray(r.astype(FP8NP))


def make_in_maps(inputs):
    inputs = {k: np.asarray(v) for k, v in inputs.items()}
    w_dd = inputs["W_dec"].astype(np.float32) @ inputs["W_enc"].astype(np.float32)
    wenc8 = _pack_pairs(inputs["W_enc"].astype(np.float32), WENC_SC)
    wdec8 = _pack_pairs(inputs["W_dec"].astype(np.float32), DSC)
    wdd8 = _pack_pairs(w_dd, DSC)
    wcls16 = np.ascontiguousarray(inputs["W_cls"].astype(ml_dtypes.bfloat16))
    b_enc = np.ascontiguousarray(inputs["b_enc"].astype(np.float32))
    b_cls16 = np.ascontiguousarray(inputs["b_cls"].astype(ml_dtypes.bfloat16))
    b_dec = inputs["b_dec"].astype(np.float32)

    in_maps = []
    for i in range(NCORES):
        sl = slice(i * NL, (i + 1) * NL)
        x = inputs["x"][sl].astype(np.float32)
        o = inputs["output"][sl].astype(np.float32)
        cat = inputs["cat_labels"][sl].astype(np.float32)
        lab = inputs["labels"][sl].astype(np.int32)
        xT = np.ascontiguousarray(x.T)
        xt8 = np.vstack([
            _pack_pairs(np.ascontiguousarray(xT[:, ch * NC : (ch + 1) * NC]), 1.0)
            for ch in range(NCH)
        ])
        ot = (o.T - b_dec[:, None]) * DSC
        ot8 = np.ascontiguousarray(ot.astype(FP8NP))
        cat16 = np.ascontiguousarray(
            cat.reshape(NN, P, C).transpose(1, 0, 2).reshape(P, NN * C)
            .astype(ml_dtypes.bfloat16)
        )
        ohn = (lab[:, None] == np.arange(C)[None, :]).astype(np.float32)
        ohn16 = np.ascontiguousarray(
            ohn.reshape(NN, P, C).transpose(1, 0, 2).reshape(P, NN * C)
            .astype(ml_dtypes.bfloat16)
        )
        oht16 = np.ascontiguousarray(ohn.T.astype(ml_dtypes.bfloat16))
        in_maps.append({
            "xt8": xt8, "ot8": ot8, "wenc8": wenc8, "wdec8": wdec8,
            "wdd8": wdd8, "wcls16": wcls16, "cat16": cat16,
            "ohn16": ohn16, "oht16": oht16,
            "b_enc": b_enc, "b_cls16": b_cls16,
        })
    return in_maps


def finish(results):
    """Combine per-core outputs: add the global scalar S to each sample."""
    outs = [np.asarray(results[i]["out"], dtype=np.float32) for i in range(NCORES)]
    rec = sum(float(o[NL + 0]) for o in outs)
    lat = sum(float(o[NL + 1]) for o in outs)
    cat = sum(float(o[NL + 2]) for o in outs)
    S = (0.9 / (DSC * N_GLOBAL * T)) * rec + (0.9 / (N_GLOBAL * D)) * lat \
        - (1.0 / N_GLOBAL) * cat
    return np.concatenate([o[:NL] for o in outs]).astype(np.float32) + np.float32(S)


def kernel(**inputs):
    nc = _get_nc()
    in_maps = make_in_maps(inputs)
    res = run_bass_kernel_spmd(nc, in_maps, list(range(NCORES))).results
    return finish(res)


# revision 3
# speedup vs baseline: 1.0124x; 1.0124x over previous
"""Trainium2 Bass kernel for nn_Custom_trainer_79242146611896 — v3.

Data-parallel over N=16384 samples across 8 NeuronCores (2048/core).
v3 design (vs v2 baseline, 284us):
  - host packs x TRANSPOSED as fp8 DoubleRow pair tiles: kills all 256
    x PE-transposes (~70us PE) and 12MB/core of DMA.
  - host packs (o^T - b_dec) x16 as fp8 [T,n] tiles: mm2 emits decodedT
    so the pinball subtract is elementwise vs oT8; 12MB/core DMA saved.
  - weights pre-cast on host (fp8 DR pairs / bf16), cat_labels packed
    bf16 [128, NN*C]: no device-side CAST passes, ~7MB/core DMA saved.
    Total DMA ~10.7MB/core (was ~41.5MB).
  - collective #2 removed: each core returns its 3 loss partials in a
    padded output row; host computes the global scalar during gather.
    Kills the ~24us exposed AllReduce tail + ~15us drain.
  - one collective (seg sums+counts [C,D+1]) launched at end of phase
    A, hidden under phase B (mm2/mm4/mm3 for all chunks).
  - pinball elementwise work split across vector/gpsimd/scalar.
"""

import numpy as np
import ml_dtypes

import concourse.bass as bass
import concourse.mybir as mybir
import concourse.tile as tile
from concourse import bacc
from concourse.bass_utils import run_bass_kernel_spmd
from concourse.masks import make_identity

F32 = mybir.dt.float32
FP8 = mybir.dt.float8e4
BF16 = mybir.dt.bfloat16
I32 = mybir.dt.int32
AX = mybir.AxisListType
ALU = mybir.AluOpType
ACTF = mybir.ActivationFunctionType
DROW = mybir.MatmulPerfMode.DoubleRow

P = 128
NCORES = 8
N_GLOBAL = 16384
T = 2048
D = 512
C = 50
KEPS = 1e-7
LNEPS = float(np.log(KEPS))

NL = N_GLOBAL // NCORES   # 2048 samples per core
NT = T // P               # 16 T-tiles
ND = D // P               # 4 D-tiles
NN = NL // P              # 16 sample-tiles per core
NC = 512                  # samples per chunk
NCH = NL // NC            # 4 chunks
NSUB = NC // P            # 4 sample-tiles per chunk

WENC_SC = 32.0            # wenc fp8 scale (tanh un-scales via 1/32)
DSC = 16.0                # wdec/Wdd/o fp8 scale


def build(gps_b=True):
    nc = bacc.Bacc("TRN2", target_bir_lowering=False, debug=False, num_devices=NCORES)

    # host-packed inputs
    xt8_d = nc.dram_tensor("xt8", [NCH * NT // 2 * P, 2 * NC], FP8, kind="ExternalInput")
    ot8_d = nc.dram_tensor("ot8", [NT * P, NL], FP8, kind="ExternalInput")
    wenc8_d = nc.dram_tensor("wenc8", [NT // 2 * P, 2 * D], FP8, kind="ExternalInput")
    wdec8_d = nc.dram_tensor("wdec8", [ND // 2 * P, 2 * T], FP8, kind="ExternalInput")
    wdd8_d = nc.dram_tensor("wdd8", [ND // 2 * P, 2 * D], FP8, kind="ExternalInput")
    wcls_d = nc.dram_tensor("wcls16", [D, C], BF16, kind="ExternalInput")
    cat_d = nc.dram_tensor("cat16", [P, NN * C], BF16, kind="ExternalInput")
    ohn_d = nc.dram_tensor("ohn16", [P, NN * C], BF16, kind="ExternalInput")
    oht_d = nc.dram_tensor("oht16", [C, NL], BF16, kind="ExternalInput")
    benc_d = nc.dram_tensor("b_enc", [D], F32, kind="ExternalInput")
    bcls_d = nc.dram_tensor("b_cls16", [C], BF16, kind="ExternalInput")
    out_d = nc.dram_tensor("out", [NL + P], F32, kind="ExternalOutput")

    from contextlib import ExitStack

    with tile.TileContext(nc) as tc:
        with ExitStack() as ctx:
            ent = ctx.enter_context
            constp = ent(tc.tile_pool(name="const", bufs=1))
            wts = ent(tc.tile_pool(name="wts", bufs=1))
            bigp = ent(tc.tile_pool(name="big", bufs=1))
            encp = ent(tc.tile_pool(name="enc", bufs=1))
            accp = ent(tc.tile_pool(name="acc", bufs=1))
            enp = ent(tc.tile_pool(name="enp", bufs=6))
            junkp = ent(tc.tile_pool(name="junk", bufs=4))
            ccep = ent(tc.tile_pool(name="cce", bufs=4))
            smallp = ent(tc.tile_pool(name="small", bufs=10))
            psA = ent(tc.tile_pool(name="psA", bufs=2, space="PSUM"))
            psT = ent(tc.tile_pool(name="psT", bufs=2, space="PSUM"))
            psS = ent(tc.tile_pool(name="psS", bufs=2, space="PSUM"))
            psSeg = ent(tc.tile_pool(name="psSeg", bufs=1, space="PSUM"))
            psCnt = ent(tc.tile_pool(name="psCnt", bufs=1, space="PSUM"))
            dp = ent(tc.tile_pool(name="dram", bufs=1, space="DRAM"))

            # ------- big input slabs: phase-A-critical DMAs first -------
            wenc8 = wts.tile([P, NT // 2 * 2 * D], FP8)
            nc.sync.dma_start(
                wenc8[:].rearrange("p (a m) -> p a m", a=NT // 2),
                wenc8_d.ap().rearrange("(a p) m -> p a m", p=P),
            )
            # xt8 chunk-major: one 1MB DMA per sample-chunk so mm1 on
            # chunk 0 starts as soon as the first quarter lands.
            xt8 = bigp.tile([P, NCH * NT // 2 * 2 * NC], FP8)
            xt8_src = xt8_d.ap().rearrange("(c a p) m -> c p a m", p=P, c=NCH)
            CW = NT // 2 * 2 * NC
            for ch in range(NCH):
                nc.sync.dma_start(
                    xt8[:, ch * CW : (ch + 1) * CW].rearrange(
                        "p (a m) -> p a m", a=NT // 2
                    ),
                    xt8_src[ch],
                )
            bencT = wts.tile([P, ND], F32)
            nc.sync.dma_start(bencT[:], benc_d.ap().rearrange("(a p) -> p a", p=P))
            ohnb = bigp.tile([P, NN * C], BF16)
            nc.sync.dma_start(ohnb[:], ohn_d.ap())
            ohtb = bigp.tile([C, NL], BF16)
            nc.sync.dma_start(ohtb[:], oht_d.ap())
            bcls_row_t = wts.tile([1, C], BF16)
            bcls_row = bcls_row_t[0:1, :]
            nc.sync.dma_start(bcls_row, bcls_d.ap().rearrange("(o c) -> o c", o=1))
            wclsb = wts.tile([P, ND * C], BF16)
            nc.sync.dma_start(
                wclsb[:].rearrange("p (a c) -> p a c", a=ND),
                wcls_d.ap().rearrange("(a p) c -> p a c", p=P),
            )
            catb = bigp.tile([P, NN * C], BF16)
            nc.sync.dma_start(catb[:], cat_d.ap())
            wdd8 = wts.tile([P, ND // 2 * 2 * D], FP8)
            nc.sync.dma_start(
                wdd8[:].rearrange("p (a m) -> p a m", a=ND // 2),
                wdd8_d.ap().rearrange("(a p) m -> p a m", p=P),
            )
            wdec8 = wts.tile([P, ND // 2 * 2 * T], FP8)
            nc.sync.dma_start(
                wdec8[:].rearrange("p (a m) -> p a m", a=ND // 2),
                wdec8_d.ap().rearrange("(a p) m -> p a m", p=P),
            )
            # ot8 last: the sync HWDGE ring drains FIFO, so emission
            # order prioritizes the phase-A-critical loads above.
            ot8 = bigp.tile([P, NT * NL], FP8)
            nc.sync.dma_start(
                ot8[:].rearrange("p (a m) -> p a m", a=NT),
                ot8_d.ap().rearrange("(a p) m -> p a m", p=P),
            )

            # ---------------- constants ----------------
            ident_f32 = constp.tile([P, P], F32)
            make_identity(nc, ident_f32)
            ident_bf = constp.tile([P, P], BF16)
            nc.vector.tensor_copy(ident_bf[:], ident_f32[:])
            nident8 = constp.tile([P, P], FP8)
            nc.scalar.activation(nident8[:], ident_f32[:], mybir.ActivationFunctionType.Copy, scale=-1.0)
            ones_col_f = constp.tile([P, 1], F32)
            nc.any.memset(ones_col_f[:], 1.0)
            ones_col_bf = constp.tile([P, 1], BF16)
            nc.vector.tensor_copy(ones_col_bf[:], ones_col_f[:])
            ones_k1bf = constp.tile([1, P], BF16)
            nc.any.memset(ones_k1bf[:], 1.0)

            # pre-touch the tanh table during the DMA wait
            dummy = smallp.tile([P, 1], F32, name="dummy", tag="dummy")
            nc.scalar.activation(dummy[:], ones_col_f[:], ACTF.Tanh)

            # ---------------- persistent state ----------------
            encT = [encp.tile([P, NL], BF16, name=f"encT{k}", tag=f"encT{k}") for k in range(ND)]
            encT8 = [encp.tile([P, 2 * NL], FP8, name=f"encT8_{kp}", tag=f"encT8_{kp}") for kp in range(ND // 2)]
            rec_strip = accp.tile([P, NCH * NT], F32)
            lat_strip = accp.tile([P, NCH * ND], F32)
            cat_strip = accp.tile([P, NN], F32)
            nsq_strip = accp.tile([P, NN], F32)

            seg_ps = psSeg.tile([C, D], F32, name="seg_ps", tag="seg")
            cnt_ps = psCnt.tile([C, 1], F32, name="cnt_ps", tag="cnt")

            xt8_r = xt8[:].rearrange(
                "p (c a two n) -> p c a two n", c=NCH, a=NT // 2, two=2
            )
            wenc8_r = wenc8[:].rearrange("p (a two d) -> p a two d", a=NT // 2, two=2)
            wdec8_r = wdec8[:].rearrange("p (a two t) -> p a two t", a=ND // 2, two=2)
            wdd8_r = wdd8[:].rearrange("p (a two d) -> p a two d", a=ND // 2, two=2)

            # ================= phase A: mm1 + transposes + seg =================
            for c in range(NCH):
                base = c * NC
                for k in range(ND):
                    ps = psA.tile([P, NC], F32, name="ps1", tag="psA")
                    for tp in range(NT // 2):
                        nc.tensor.matmul(
                            ps[:],
                            wenc8_r[:, tp, :, k * P : (k + 1) * P],
                            xt8_r[:, c, tp, :, :],
                            start=(tp == 0), stop=(tp == NT // 2 - 1),
                            perf_mode=DROW,
                        )
                    nc.scalar.activation(
                        encT[k][:, base : base + NC], ps[:], ACTF.Tanh,
                        bias=bencT[:, k : k + 1], scale=1.0 / WENC_SC,
                    )
                    nc.gpsimd.tensor_copy(
                        encT8[k // 2][:, (k % 2) * NL + base : (k % 2) * NL + base + NC],
                        encT[k][:, base : base + NC],
                    )

                for s in range(NSUB):
                    i = c * NSUB + s
                    pse = psT.tile([P, D], BF16, name="pseb", tag="psT")
                    for k in range(ND):
                        nc.tensor.transpose(
                            pse[:, k * P : (k + 1) * P],
                            encT[k][:, base + s * P : base + (s + 1) * P],
                            ident_bf,
                        )
                    en = enp.tile([P, D], BF16, name="en", tag="en")
                    nc.scalar.copy(en[:], pse[:])

                    oh_i = ohnb[:, i * C : (i + 1) * C]
                    nc.tensor.matmul(
                        seg_ps[:], oh_i, en[:],
                        start=(i == 0), stop=(i == NN - 1),
                    )
                    nc.tensor.matmul(
                        cnt_ps[:], oh_i, ones_col_bf[:],
                        start=(i == 0), stop=(i == NN - 1),
                    )

                    jn = junkp.tile([P, D], F32, name="jn", tag="junk")
                    nc.vector.scalar_tensor_tensor(
                        out=jn[:], in0=en[:], scalar=0.0, in1=en[:],
                        op0=ALU.bypass, op1=ALU.mult,
                        accum_out=nsq_strip[:, i : i + 1],
                    )

            # ============ collective: seg sums + counts ============
            seg_sb = accp.tile([C, D], F32)
            nc.vector.tensor_copy(seg_sb[:], seg_ps[:])
            cnt_sb = accp.tile([C, 1], F32)
            nc.vector.tensor_copy(cnt_sb[:], cnt_ps[:])

            bounce1_in = dp.tile([C, D + 1], F32, name="bounce1_in", tag="b1i")
            bounce1_out = dp.tile([C, D + 1], F32, name="bounce1_out", tag="b1o")
            nc.sync.dma_start(bounce1_in[:, 0:D], seg_sb[:])
            nc.sync.dma_start(bounce1_in[:, D : D + 1], cnt_sb[:])
            nc.gpsimd.collective_compute(
                "AllReduce",
                ALU.add,
                replica_groups=[list(range(NCORES))],
                ins=[bounce1_in[:].opt()],
                outs=[bounce1_out[:].opt()],
            )

            # ================= phase B (interleaved per chunk: B1's
            # matmuls cover B2/B3's vector+scalar work) =================
            for c in range(NCH):
                base = c * NC
                # B1: decodedT per T-tile + rec pinball (diff is 16x scaled;
                # host coefficient divides by 16). Odd tiles subtract o on
                # the PE via a -I @ oT8 accumulation matmul.
                for t in range(NT):
                    pool = (psA, psT, psS)[t % 3]
                    ps = pool.tile([P, NC], F32, name="ps2", tag=pool.name)
                    for kp in range(ND // 2):
                        nc.tensor.matmul(
                            ps[:],
                            wdec8_r[:, kp, :, t * P : (t + 1) * P],
                            encT8[kp].rearrange("p (two n) -> p two n", two=2)[
                                :, :, base : base + NC
                            ],
                            start=(kp == 0),
                            stop=False,
                            perf_mode=DROW,
                        )
                    nc.tensor.matmul(
                        ps[:], nident8[:],
                        ot8[:, t * NL + base : t * NL + base + NC],
                        start=False, stop=True,
                    )
                    col = rec_strip[:, c * NT + t : c * NT + t + 1]
                    if t % 4 < 2:
                        nc.scalar.activation(ps[:], ps[:], ACTF.Abs, accum_out=col)
                    else:
                        nc.vector.tensor_reduce(
                            col, ps[:], AX.X, ALU.add, apply_absolute_value=True,
                        )

                # B2: rec_latentsT via W_dd + lat pinball
                for k2 in range(ND):
                    pool = (psA, psT, psS)[(NT + k2) % 3]
                    ps = pool.tile([P, NC], F32, name="ps4", tag=pool.name)
                    for kp in range(ND // 2):
                        nc.tensor.matmul(
                            ps[:],
                            wdd8_r[:, kp, :, k2 * P : (k2 + 1) * P],
                            encT8[kp].rearrange("p (two n) -> p two n", two=2)[
                                :, :, base : base + NC
                            ],
                            start=(kp == 0), stop=(kp == ND // 2 - 1),
                            perf_mode=DROW,
                        )
                    nc.scalar.activation(
                        ps[:], ps[:], ACTF.Tanh, bias=bencT[:, k2 : k2 + 1], scale=1.0 / DSC
                    )
                    nc.vector.tensor_tensor(
                        ps[:], ps[:], encT[k2][:, base : base + NC], ALU.subtract
                    )
                    col = lat_strip[:, c * ND + k2 : c * ND + k2 + 1]
                    nc.vector.tensor_reduce(
                        col, ps[:], AX.X, ALU.add, apply_absolute_value=True,
                    )

                # B3: logits + CCE
                psl = psS.tile([P, NSUB * C], F32, name="psl", tag="psS")
                for a in range(NSUB):
                    sl = slice(a * C, (a + 1) * C)
                    nc.tensor.matmul(
                        psl[:, sl], ones_k1bf[:], bcls_row[:],
                        start=True, stop=False,
                    )
                    for k in range(ND):
                        nc.tensor.matmul(
                            psl[:, sl],
                            encT[k][:, (base + a * P) : (base + (a + 1) * P)],
                            wclsb[:, k * C : (k + 1) * C],
                            start=False, stop=(k == ND - 1),
                        )
                expt = ccep.tile([P, NSUB * C], F32, name="expt", tag="cce", bufs=2)
                nc.scalar.activation(expt[:], psl[:], ACTF.Exp)
                sume = smallp.tile([P, NSUB], F32, name="sume", tag="small")
                nc.vector.tensor_reduce(
                    sume[:], expt[:].rearrange("p (a c) -> p a c", c=C),
                    AX.X, ALU.add,
                )
                rcp = smallp.tile([P, NSUB], F32, name="rcp", tag="small")
                nc.vector.reciprocal(rcp[:], sume[:])
                clsl = catb[:, c * NSUB * C : (c + 1) * NSUB * C]
                rs = smallp.tile([P, NSUB], F32, name="rs", tag="small")
                nc.vector.tensor_reduce(
                    rs[:], clsl.rearrange("p (a c) -> p a c", c=C),
                    AX.X, ALU.add,
                )
                lnrs = smallp.tile([P, NSUB], F32, name="lnrs", tag="small")
                nc.scalar.activation(lnrs[:], rs[:], ACTF.Ln)
                lg = ccep.tile([P, NSUB * C], F32, name="lg", tag="lgbuf", bufs=2)
                nc.scalar.activation(lg[:], clsl, ACTF.Ln)
                for a in range(NSUB):
                    i = c * NSUB + a
                    sl = slice(a * C, (a + 1) * C)
                    lgc = ccep.tile([P, C], F32, name="lgc", tag="lgc")
                    nc.vector.tensor_scalar(
                        out=lgc[:], in0=lg[:, sl], scalar1=lnrs[:, a : a + 1],
                        scalar2=LNEPS, op0=ALU.subtract, op1=ALU.max,
                    )
                    jc = ccep.tile([P, C], F32, name="jc", tag="lgc")
                    nc.vector.scalar_tensor_tensor(
                        out=jc[:], in0=expt[:, sl], scalar=rcp[:, a : a + 1],
                        in1=lgc[:], op0=ALU.mult, op1=ALU.mult,
                        accum_out=cat_strip[:, i : i + 1],
                    )

            # readback of collective (late emission keeps the DMA queue clear)
            sums_g = accp.tile([C, D], F32)
            nc.sync.dma_start(sums_g[:], bounce1_out[:, 0:D])
            counts_g = accp.tile([C, 1], F32)
            nc.sync.dma_start(counts_g[:], bounce1_out[:, D : D + 1])

            # ================= phase C =================
            cmax = accp.tile([C, 1], F32)
            nc.vector.tensor_scalar(
                out=cmax[:], in0=counts_g[:], scalar1=1.0, scalar2=None, op0=ALU.max
            )
            crcp = accp.tile([C, 1], F32)
            nc.vector.reciprocal(crcp[:], cmax[:])
            means = accp.tile([C, D], F32)
            nc.vector.tensor_scalar(
                out=means[:], in0=sums_g[:], scalar1=crcp[:], scalar2=None, op0=ALU.mult
            )
            msq_col = accp.tile([C, 1], F32)
            jm = junkp.tile([C, D], F32, name="jm", tag="junk")
            nc.vector.scalar_tensor_tensor(
                out=jm[:], in0=means[:], scalar=0.0, in1=means[:],
                op0=ALU.bypass, op1=ALU.mult, accum_out=msq_col[:],
            )

            meansT = []
            for k in range(ND):
                tpm = psS.tile([P, C], F32, name="tpm", tag="psS")
                nc.tensor.transpose(
                    tpm[:], means[:, k * P : (k + 1) * P], ident_f32[:C, :C]
                )
                mt = accp.tile([P, C], BF16, name="meansT", tag=f"meansT{k}")
                nc.vector.tensor_copy(mt[:], tpm[:])
                meansT.append(mt)

            # epsT[c,n] = sum_d means[c,d] enc[n,d]  (transposed layout);
            # jq2 = (epsT - 0.5*msq_col) * ohT ; gq_row[n] = colsum(jq2)
            msqh = accp.tile([C, 1], F32)
            nc.vector.tensor_scalar(
                out=msqh[:], in0=msq_col[:], scalar1=0.5, scalar2=None, op0=ALU.mult
            )
            gq_cols = psT.tile([P, NN], F32, name="gq_cols", tag="psT")
            for c in range(NCH):
                base = c * NC
                pscT = psS.tile([C, NC], F32, name="pscT", tag="psS")
                for k in range(ND):
                    nc.tensor.matmul(
                        pscT[:],
                        meansT[k][:],
                        encT[k][:, base : base + NC],
                        start=(k == 0), stop=(k == ND - 1),
                    )
                jq2 = ccep.tile([C, NC], F32, name="jq2", tag="jq2", bufs=2)
                nc.vector.scalar_tensor_tensor(
                    out=jq2[:], in0=pscT[:], scalar=msqh[:], in1=ohtb[:, base : base + NC],
                    op0=ALU.subtract, op1=ALU.mult,
                )
                gq_ps = psA.tile([1, NC], F32, name="gq_ps", tag="psA")
                nc.tensor.matmul(
                    gq_ps[:], ones_col_f[0:C, :], jq2[:], start=True, stop=True
                )
                gq_row = smallp.tile([1, NC], F32, name="gq_row", tag=f"gqr{c}", bufs=1)
                nc.vector.tensor_copy(gq_row[:], gq_ps[:])
                for s in range(NSUB):
                    i = c * NSUB + s
                    nc.tensor.transpose(
                        gq_cols[:, i : i + 1],
                        gq_row[:, s * P : (s + 1) * P],
                        ident_f32[0:1, 0:1],
                    )

            # ---- final: (nsq - 2*gq)/D per sample; pack partial scalars ----
            ns2 = accp.tile([P, NN], F32)
            nc.vector.tensor_scalar(
                out=ns2[:], in0=nsq_strip[:], scalar1=1.0 / D, scalar2=None, op0=ALU.mult
            )
            out_strip = accp.tile([P, NN], F32)
            nc.vector.scalar_tensor_tensor(
                out=out_strip[:], in0=gq_cols[:], scalar=-2.0 / D, in1=ns2[:],
                op0=ALU.mult, op1=ALU.add,
            )
            pack3 = accp.tile([P, 3], F32)
            nc.vector.tensor_reduce(pack3[:, 0:1], rec_strip[:], AX.X, ALU.add)
            nc.vector.tensor_reduce(pack3[:, 1:2], lat_strip[:], AX.X, ALU.add)
            nc.vector.tensor_reduce(pack3[:, 2:3], cat_strip[:], AX.X, ALU.add)
            scps = psS.tile([1, 3], F32, name="scps", tag="psS")
            nc.tensor.matmul(scps[:], ones_col_f[:], pack3[:], start=True, stop=True)

            ps_out = psS.tile([NN, P], F32, name="ps_out", tag="psS")
            nc.tensor.transpose(ps_out[:], out_strip[:], ident_f32)
            outT = accp.tile([NN, P], F32)
            nc.vector.tensor_copy(outT[:], ps_out[:])
            sc_row = accp.tile([1, P], F32)
            nc.any.memset(sc_row[:], 0.0)
            nc.vector.tensor_copy(sc_row[0:1, 0:3], scps[:])
            nc.sync.dma_start(
                out_d.ap()[0:NL].rearrange("(a p) -> a p", p=P), outT[:]
            )
            nc.sync.dma_start(
                out_d.ap()[NL : NL + P].rearrange("(o p) -> o p", o=1), sc_row[:]
            )

    nc.compile()
    return nc


_CACHE = {}


def _get_nc():
    if "nc" not in _CACHE:
        _CACHE["nc"] = build()
    return _CACHE["nc"]


FP8NP = ml_dtypes.float8_e4m3


def _pack_pairs(w, scale):
    """[K, M] f32 -> [K//256 * 128, 2*M] fp8 DoubleRow pair tiles."""
    K, M = w.shape
    a = K // (2 * P)
    r = (w * scale).reshape(a, 2, P, M).transpose(0, 2, 1, 3).reshape(a * P, 2 * M)
    return np.ascontiguousarray(r.astype(FP8NP))


def make_in_maps(inputs):
    inputs = {k: np.asarray(v) for k, v in inputs.items()}
    w_dd = inputs["W_dec"].astype(np.float32) @ inputs["W_enc"].astype(np.float32)
    wenc8 = _pack_pairs(inputs["W_enc"].astype(np.float32), WENC_SC)
    wdec8 = _pack_pairs(inputs["W_dec"].astype(np.float32), DSC)
    wdd8 = _pack_pairs(w_dd, DSC)
    wcls16 = np.ascontiguousarray(inputs["W_cls"].astype(ml_dtypes.bfloat16))
    b_enc = np.ascontiguousarray(inputs["b_enc"].astype(np.float32))
    b_cls16 = np.ascontiguousarray(inputs["b_cls"].astype(ml_dtypes.bfloat16))
    b_dec = inputs["b_dec"].astype(np.float32)

    in_maps = []
    for i in range(NCORES):
        sl = slice(i * NL, (i + 1) * NL)
        x = inputs["x"][sl].astype(np.float32)
        o = inputs["output"][sl].astype(np.float32)
        cat = inputs["cat_labels"][sl].astype(np.float32)
        lab = inputs["labels"][sl].astype(np.int32)
        xT = np.ascontiguousarray(x.T)
        xt8 = np.vstack([
            _pack_pairs(np.ascontiguousarray(xT[:, ch * NC : (ch + 1) * NC]), 1.0)
            for ch in range(NCH)
        ])
        ot = (o.T - b_dec[:, None]) * DSC
        ot8 = np.ascontiguousarray(ot.astype(FP8NP))
        cat16 = np.ascontiguousarray(
            cat.reshape(NN, P, C).transpose(1, 0, 2).reshape(P, NN * C)
            .astype(ml_dtypes.bfloat16)
        )
        ohn = (lab[:, None] == np.arange(C)[None, :]).astype(np.float32)
        ohn16 = np.ascontiguousarray(
            ohn.reshape(NN, P, C).transpose(1, 0, 2).reshape(P, NN * C)
            .astype(ml_dtypes.bfloat16)
        )
        oht16 = np.ascontiguousarray(ohn.T.astype(ml_dtypes.bfloat16))
        in_maps.append({
            "xt8": xt8, "ot8": ot8, "wenc8": wenc8, "wdec8": wdec8,
            "wdd8": wdd8, "wcls16": wcls16, "cat16": cat16,
            "ohn16": ohn16, "oht16": oht16,
            "b_enc": b_enc, "b_cls16": b_cls16,
        })
    return in_maps


def finish(results):
    """Combine per-core outputs: add the global scalar S to each sample."""
    outs = [np.asarray(results[i]["out"], dtype=np.float32) for i in range(NCORES)]
    rec = sum(float(o[NL + 0]) for o in outs)
    lat = sum(float(o[NL + 1]) for o in outs)
    cat = sum(float(o[NL + 2]) for o in outs)
    S = (0.9 / (DSC * N_GLOBAL * T)) * rec + (0.9 / (N_GLOBAL * D)) * lat \
        - (1.0 / N_GLOBAL) * cat
    return np.concatenate([o[:NL] for o in outs]).astype(np.float32) + np.float32(S)


def kernel(**inputs):
    nc = _get_nc()
    in_maps = make_in_maps(inputs)
    res = run_bass_kernel_spmd(nc, in_maps, list(range(NCORES))).results
    return finish(res)
